# revision 15
# baseline (speedup 1.0000x reference)
"""Trainium2 Bass kernel for a 3-layer shared-weight LSTM (CharRNN).

Math (per batch row):
    for t: 3 stacked LSTM cells with shared (W, U, b); top h -> Dense(Wd, bd)

Strategy:
  - Data-parallel over batch: B=50 padded to 56 = 8 cores x 7 rows.
  - Per core a single sequential wavefront over s = 0..T+1 processes
    (layer0, t=s), (layer1, t=s-1), (layer2, t=s-2) together, so the
    sequential critical path is T+2 steps instead of 3*T.
  - Feature-major layout [65 units x 21 lanes] (21 = 3 layers x 7 rows);
    gates land in one PSUM tile Z[65, 84] with gate order (i, f, o, g) via
    12 tiny PE matmuls per step; g-columns of the weights are pre-scaled
    by 2 so a single Sigmoid over all 84 columns also yields
    tanh(g) = 2*sigmoid(2g) - 1 (fixed up by a fused scalar_tensor_tensor).
  - Cell update is 4 DVE ops; tanh(c) on the scalar engine; the h-write
    lands directly in the next step's matmul rhs (H buffer [h0 | h1 | h2],
    layer inputs and recurrent reads are overlapping windows of it).
  - Bias enters through an extra ones-row in the contraction (row 65 of
    xT and of H).
  - Top-layer h is staged 16 timesteps at a time; Dense is one PE matmul
    per 16 steps, copied PSUM->SBUF and DMA'd to the output.

The host pre-permutes/scales the weights and pre-transposes x into the
feature-major layout (pure input marshalling), and gathers the shards.
"""

import sys

if "/opt/trn_rl_repo" not in sys.path:
    sys.path.insert(0, "/opt/trn_rl_repo")

import numpy as np

UNITS = 65
NCORES = 8
BP = 7           # batch rows per core (50 -> pad 56)
T_FULL = 2048
CHUNK_T = 16     # timesteps per dense/output chunk


def _build_program(T: int, Bp: int):
    from contextlib import ExitStack

    import concourse.bacc as bacc
    import concourse.bass as bass  # noqa: F401
    import concourse.mybir as mybir
    import concourse.tile as tile
    from concourse.tile_rust import add_dep_helper

    f32 = mybir.dt.float32
    AF = mybir.ActivationFunctionType
    ALU = mybir.AluOpType

    S = T + 2            # wavefront steps
    NB = 3 * Bp          # wavefront width (3 layers x Bp)
    W4 = 4 * NB          # four gates

    nc = bacc.Bacc(None, target_bir_lowering=False)
    xT_d = nc.dram_tensor("xT", [66, Bp * S], f32, kind="ExternalInput")
    # WALL packs [WXb (66x260) | U-perm (65x260, row65=0) | WD (66x65)]
    WALL_d = nc.dram_tensor("WALL", [66, 585], f32, kind="ExternalInput")
    y_d = nc.dram_tensor("y", [Bp, T, UNITS], f32, kind="ExternalOutput")

    with tile.TileContext(nc) as tc:
        with ExitStack() as ctx:
            const = ctx.enter_context(tc.tile_pool(name="const", bufs=1))
            work = ctx.enter_context(tc.tile_pool(name="work", bufs=3))
            zp = ctx.enter_context(tc.tile_pool(name="zp", bufs=2, space="PSUM"))
            yp = ctx.enter_context(tc.tile_pool(name="yp", bufs=2, space="PSUM"))

            # --- static data ---
            xT = const.tile([66, Bp * S], f32)
            nc.sync.dma_start(xT[:], xT_d[:])
            WALL = const.tile([66, 585], f32)
            nc.sync.dma_start(WALL[:], WALL_d[:])

            def WX(g):
                return WALL[:, UNITS * g:UNITS * (g + 1)]

            def UU(g):
                return WALL[0:65, 260 + UNITS * g:260 + UNITS * (g + 1)]

            WD = WALL[:, 520:585]

            # --- state (manually double-buffered persistent tiles) ---
            # H columns: [h0 | h1 | h2]; row 65 is the bias ones-row.
            H = [const.tile([66, NB], f32, name=f"H{i}") for i in range(2)]
            Cst = [const.tile([65, NB], f32, name=f"Cst{i}") for i in range(2)]
            stage = [const.tile([66, Bp * CHUNK_T], f32, name=f"stage{i}")
                     for i in range(2)]

            for i in range(2):
                # engines need quadrant-aligned partition starts: set rows
                # 64:66 to one first, then zero rows 0:65 (row 65 survives)
                nc.vector.memset(H[i][64:66, :], 1.0)
                nc.vector.memset(H[i][0:65, :], 0.0)
                nc.vector.memset(Cst[i][:, :], 0.0)
                nc.vector.memset(stage[i][64:66, :], 1.0)

            for s in range(S):
                cur = s % 2
                nxt = (s + 1) % 2
                Hc, Hn = H[cur], H[nxt]
                Cc, Cn = Cst[cur], Cst[nxt]

                # One PSUM accumulation group per step over the whole Z
                # bank: the first matmul carries start (marks the bank
                # pending-zero; first write per byte overwrites), the last
                # carries stop.  A no-sync chain pins the PE order.
                Z = zp.tile([65, W4], f32, name="Z")
                mms = []
                for g in range(4):
                    # layer-0 input term: x_t (static, no sync needed)
                    mms.append(nc.tensor.matmul(
                        Z[:, NB * g:NB * g + Bp],
                        WX(g),
                        xT[:, Bp * s:Bp * (s + 1)],
                        start=(g == 0), stop=False,
                        skip_group_check=False,
                    ))
                for g in range(4):
                    # layer-1/2 input term: h0, h1 from previous step
                    mms.append(nc.tensor.matmul(
                        Z[:, NB * g + Bp:NB * g + NB],
                        WX(g),
                        Hc[:, 0:2 * Bp],
                        start=False, stop=False,
                        skip_group_check=False,
                    ))
                    # recurrent term for all three layers
                    mms.append(nc.tensor.matmul(
                        Z[:, NB * g:NB * g + NB],
                        UU(g),
                        Hc[0:65, 0:NB],
                        start=False, stop=(g == 3),
                        skip_group_check=False,
                    ))
                for a, bb_ in zip(mms[1:], mms[:-1]):
                    add_dep_helper(a.ins, bb_.ins, False, "psum group order")

                Sg = work.tile([65, W4], f32, name="Sg")
                nc.scalar.activation(Sg[:], Z[:], AF.Sigmoid)

                M1 = work.tile([65, NB], f32, name="M1")
                nc.vector.scalar_tensor_tensor(
                    M1[:], Sg[:, 3 * NB:4 * NB], -0.5, Sg[:, 0:NB],
                    ALU.add, ALU.mult,
                )
                M2 = work.tile([65, NB], f32, name="M2")
                nc.vector.tensor_mul(M2[:], Sg[:, NB:2 * NB], Cc[:])
                nc.vector.scalar_tensor_tensor(
                    Cn[:], M1[:], 2.0, M2[:], ALU.mult, ALU.add,
                )
                T2 = work.tile([65, NB], f32, name="T2")
                nc.scalar.activation(T2[:], Cn[:], AF.Tanh)
                nc.vector.tensor_mul(
                    Hn[0:65, 0:NB], T2[:], Sg[:, 2 * NB:3 * NB],
                )

                # Wavefront warm-up: if b != 0 the not-yet-active upper
                # layers compute garbage from the bias alone; re-zero them.
                if s == 0:
                    nc.vector.memset(Cn[:, Bp:NB], 0.0)
                    nc.vector.memset(Hn[0:65, Bp:NB], 0.0)
                if s == 1:
                    nc.vector.memset(Cn[:, 2 * Bp:NB], 0.0)
                    nc.vector.memset(Hn[0:65, 2 * Bp:NB], 0.0)

                # stage top-layer h (timestep t = s - 2) and run Dense per chunk
                if s >= 2:
                    t = s - 2
                    c = t // CHUNK_T
                    tp = t % CHUNK_T
                    st = stage[c % 2]
                    nc.gpsimd.tensor_copy(
                        st[0:65, Bp * tp:Bp * (tp + 1)],
                        Hn[0:65, 2 * Bp:NB],
                    )
                    if tp == CHUNK_T - 1 or t == T - 1:
                        nt = tp + 1
                        rows = Bp * nt
                        yps = yp.tile([Bp * CHUNK_T, UNITS], f32, name="yps")
                        nc.tensor.matmul(
                            yps[0:rows, :], st[:, 0:rows], WD,
                            start=True, stop=True,
                        )
                        ysb = work.tile([Bp * CHUNK_T, UNITS], f32, name="ysb")
                        nc.scalar.copy(ysb[0:rows, :], yps[0:rows, :])
                        nc.sync.dma_start(
                            y_d[:, CHUNK_T * c:CHUNK_T * c + nt, :]
                            .rearrange("b t d -> t b d"),
                            ysb[0:rows, :],
                        )
    nc.finalize()
    return nc


def _prep_weights(W, U, b, Wd, bd):
    """Permute gates (i,f,g,o) -> (i,f,o,g), scale g-columns by 2, fold
    biases into an extra contraction row; pack into one [66, 585] tensor."""
    perm = np.concatenate([np.arange(0, 65), np.arange(65, 130),
                           np.arange(195, 260), np.arange(130, 195)])
    gscale = np.concatenate([np.ones(195, np.float32),
                             np.full(65, 2.0, np.float32)])
    Wp = (W[:, perm] * gscale).astype(np.float32)
    Up = (U[:, perm] * gscale).astype(np.float32)
    bp = (b[perm] * gscale).astype(np.float32)
    WALL = np.zeros((66, 585), np.float32)
    WALL[0:65, 0:260] = Wp
    WALL[65, 0:260] = bp
    WALL[0:65, 260:520] = Up
    WALL[0:65, 520:585] = Wd.astype(np.float32)
    WALL[65, 520:585] = bd.astype(np.float32)
    return np.ascontiguousarray(WALL)


_PROG = None

# test-harness knobs (harness calls kernel() with defaults)
TRACE = False
TRACE_KWARGS = {}
LAST_RESULT = None


def _get_program():
    global _PROG
    if _PROG is None:
        _PROG = _build_program(T_FULL, BP)
    return _PROG


def kernel(x, W, U, b, Wd, bd):
    from concourse.bass_utils import run_bass_kernel_spmd

    x = np.asarray(x, np.float32)
    B, T, D = x.shape
    assert (T, D) == (T_FULL, UNITS)

    WALL = _prep_weights(
        np.asarray(W, np.float32), np.asarray(U, np.float32),
        np.asarray(b, np.float32), np.asarray(Wd, np.float32),
        np.asarray(bd, np.float32),
    )

    S = T + 2
    xpad = np.zeros((NCORES * BP, T, D), np.float32)
    xpad[:B] = x

    in_maps = []
    for c in range(NCORES):
        xs = xpad[c * BP:(c + 1) * BP]
        xTc = np.zeros((66, BP * S), np.float32)
        xTc[65, :] = 1.0
        xTc[0:65, 0:T * BP] = xs.transpose(1, 0, 2).reshape(T * BP, 65).T
        in_maps.append({"xT": np.ascontiguousarray(xTc), "WALL": WALL})

    nc = _get_program()
    res = run_bass_kernel_spmd(nc, in_maps, list(range(NCORES)),
                               trace=TRACE, **TRACE_KWARGS)
    global LAST_RESULT
    LAST_RESULT = res
    y = np.concatenate([np.asarray(res.results[c]["y"])
                        for c in range(NCORES)], axis=0)[:B]
    return np.ascontiguousarray(y.astype(np.float32))


# revision 37
# speedup vs baseline: 1.8069x; 1.8069x over previous
"""Trainium2 Bass kernel for a 3-layer shared-weight LSTM (CharRNN).

Math (per batch row):
    for t: 3 stacked LSTM cells with shared (W, U, b); top h -> Dense(Wd, bd)

Strategy:
  - Data-parallel over batch: B=50 padded to 56 = 8 cores x 7 rows.
  - Per core a single sequential wavefront over s = 0..T+1 processes
    (layer0, t=s), (layer1, t=s-1), (layer2, t=s-2) together, so the
    sequential critical path is T+2 steps instead of 3*T.
  - Feature-major layout [65 units x 21 lanes] (21 = 3 layers x 7 rows);
    gates land in one PSUM tile Z[65, 84] with gate order (i, f, o, g) via
    12 tiny PE matmuls per step; g-columns of the weights are pre-scaled
    by 2 so a single Sigmoid over all 84 columns also yields
    tanh(g) = 2*sigmoid(2g) - 1 (fixed up by a fused scalar_tensor_tensor).
  - Cell update is 4 DVE ops; tanh(c) on the scalar engine; the h-write
    lands directly in the next step's matmul rhs (H buffer [h0 | h1 | h2],
    layer inputs and recurrent reads are overlapping windows of it).
  - Bias enters through an extra ones-row in the contraction (row 65 of
    xT and of H).
  - Top-layer h is staged 16 timesteps at a time; Dense is one PE matmul
    per 16 steps, copied PSUM->SBUF and DMA'd to the output.

The host pre-permutes/scales the weights and pre-transposes x into the
feature-major layout (pure input marshalling), and gathers the shards.
"""

import sys

if "/opt/trn_rl_repo" not in sys.path:
    sys.path.insert(0, "/opt/trn_rl_repo")

import numpy as np

UNITS = 65
NCORES = 8
BP = 7           # batch rows per core (50 -> pad 56)
T_FULL = 2048
CHUNK_T = 16     # timesteps per dense/output chunk


def _build_program(T: int, Bp: int):
    from contextlib import ExitStack

    import concourse.bacc as bacc
    import concourse.bass as bass  # noqa: F401
    import concourse.mybir as mybir
    import concourse.tile as tile
    from concourse.tile_rust import add_dep_helper

    f32 = mybir.dt.float32
    bf16 = mybir.dt.bfloat16
    AF = mybir.ActivationFunctionType
    ALU = mybir.AluOpType

    S = T + 2            # wavefront steps
    NB = 3 * Bp          # wavefront width (3 layers x Bp)
    W4 = 4 * NB          # four gates

    nc = bacc.Bacc(None, target_bir_lowering=False)
    xT_d = nc.dram_tensor("xT", [66, Bp * S], bf16, kind="ExternalInput")
    # WALL packs [WXb (66x260) | U-perm (65x260, row65=0) | WD (66x65)]
    WALL_d = nc.dram_tensor("WALL", [66, 585], bf16, kind="ExternalInput")
    y_d = nc.dram_tensor("y", [Bp, T, UNITS], f32, kind="ExternalOutput")

    with tile.TileContext(nc) as tc:
        with ExitStack() as ctx:
            const = ctx.enter_context(tc.tile_pool(name="const", bufs=1))
            work = ctx.enter_context(tc.tile_pool(name="work", bufs=3))
            zp = ctx.enter_context(tc.tile_pool(name="zp", bufs=2, space="PSUM"))
            yp = ctx.enter_context(tc.tile_pool(name="yp", bufs=2, space="PSUM"))
            cp = ctx.enter_context(tc.tile_pool(name="cp", bufs=2, space="PSUM"))

            # --- static data ---
            xT = const.tile([66, Bp * S], bf16)
            nc.sync.dma_start(xT[:], xT_d[:])
            WALL = const.tile([66, 585], bf16)
            nc.sync.dma_start(WALL[:], WALL_d[:])

            def WX(g):
                return WALL[:, UNITS * g:UNITS * (g + 1)]

            def UU(g):
                return WALL[0:65, 260 + UNITS * g:260 + UNITS * (g + 1)]

            WD = WALL[:, 520:585]

            # --- state (manually double-buffered persistent tiles) ---
            # H columns: [h0 | h1 | h2]; row 65 is the bias ones-row.
            H = [const.tile([66, NB], bf16, name=f"H{i}") for i in range(2)]
            stage = [const.tile([66, Bp * CHUNK_T], bf16, name=f"stage{i}")
                     for i in range(2)]

            for i in range(2):
                # engines need quadrant-aligned partition starts: set rows
                # 64:66 to one first, then zero rows 0:65 (row 65 survives)
                nc.vector.memset(H[i][64:66, :], 1.0)
                nc.vector.memset(H[i][0:65, :], 0.0)
                nc.vector.memset(stage[i][64:66, :], 1.0)
            # c lives in PSUM (cheaper tanh source); rotating pool tiles
            Cc = cp.tile([65, NB], f32, name="Cn")
            nc.vector.memset(Cc[:, :], 0.0)

            prev_v3 = None
            for s in range(S):
                cur = s % 2
                nxt = (s + 1) % 2
                Hc, Hn = H[cur], H[nxt]

                # Gates in two PSUM banks so sigma(f,g) only waits on its
                # own matmuls (PSUM deps are bank-level).  Layout per bank:
                # [first-gate 0:NB | second-gate NB:2NB]; the x-terms (cols
                # 0:Bp / NB:NB+Bp) read only the static xT, so they are
                # hoisted BEFORE the h-dependent matmuls — they execute
                # during the previous step's activation window and keep
                # the PE warm.  One accumulation group per bank.
                # bank layouts: Zfg = [f | g], Zoi = [o | i] (o first so
                # sigmoid(o) lands 4B-aligned for the 2x-mode h multiply)
                Zfg = zp.tile([65, 2 * NB], f32, name="Zfg")
                Zoi = zp.tile([65, 2 * NB], f32, name="Zoi")
                xs_ = xT[:, Bp * s:Bp * (s + 1)]
                banks = ((Zfg, 0, 1), (Zoi, 3, 2))
                mms = []
                # early x-terms (layer 0 input): one per gate.  Held back
                # (sync dep on the previous step's tanh) so they run right
                # before the h-matmuls and keep the PE p-state warm into
                # the critical phase.
                for bank, ga, gb in banks:
                    mms.append(nc.tensor.matmul(
                        bank[:, 0:Bp], WX(ga), xs_,
                        start=True, stop=False, skip_group_check=False))
                    mms.append(nc.tensor.matmul(
                        bank[:, NB:NB + Bp], WX(gb), xs_,
                        start=False, stop=False, skip_group_check=False))
                if prev_v3 is not None:
                    add_dep_helper(mms[0].ins, prev_v3.ins, True,
                                   "pe warmup timing")
                # h-dependent terms
                for bank, ga, gb in banks:
                    # layer-1/2 input terms: [h0 | h1] (+ones bias row)
                    mms.append(nc.tensor.matmul(
                        bank[:, Bp:NB], WX(ga), Hc[:, 0:2 * Bp],
                        start=False, stop=False, skip_group_check=False))
                    mms.append(nc.tensor.matmul(
                        bank[:, NB + Bp:2 * NB], WX(gb), Hc[:, 0:2 * Bp],
                        start=False, stop=False, skip_group_check=False))
                    # recurrent terms for all 3 layers
                    mms.append(nc.tensor.matmul(
                        bank[:, 0:NB], UU(ga), Hc[0:65, 0:NB],
                        start=False, stop=False, skip_group_check=False))
                    mms.append(nc.tensor.matmul(
                        bank[:, NB:2 * NB], UU(gb), Hc[0:65, 0:NB],
                        start=False, stop=True, skip_group_check=False))
                for a, bb_ in zip(mms[1:], mms[:-1]):
                    add_dep_helper(a.ins, bb_.ins, False, "psum group order")

                # ACT order: sigma(f,g) -> sigma(o,i) -> tanh(c)
                Sg = work.tile([65, 2 * NB], f32, name="Sg")
                nc.scalar.activation(Sg[:], Zfg[:], AF.Sigmoid)
                Soi = work.tile([65, 2 * NB], bf16, name="Soi")
                nc.scalar.activation(Soi[:], Zoi[:], AF.Sigmoid)

                # m2 = sigmoid(f) * c — only needs the first sigma
                M2 = work.tile([65, NB], f32, name="M2")
                nc.vector.tensor_mul(M2[:], Sg[:, 0:NB], Cc[:])
                # m1 = (sigmoid(2g) - 0.5) * sigmoid(i) = sigmoid(i)*tanh(g)/2
                M1 = work.tile([65, NB], f32, name="M1")
                nc.vector.scalar_tensor_tensor(
                    M1[:], Sg[:, NB:2 * NB], -0.5, Soi[:, NB:2 * NB],
                    ALU.add, ALU.mult,
                )
                Cn = cp.tile([65, NB], f32, name="Cn")
                prev_v3 = nc.vector.scalar_tensor_tensor(
                    Cn[:], M1[:], 2.0, M2[:], ALU.mult, ALU.add,
                )
                T2 = work.tile([65, NB], bf16, name="T2")
                nc.scalar.activation(T2[:], Cn[:], AF.Tanh)
                # h = tanh(c') * sigmoid(o)
                nc.vector.tensor_mul(
                    Hn[0:65, 0:NB], T2[:], Soi[:, 0:NB],
                )

                # Wavefront warm-up: if b != 0 the not-yet-active upper
                # layers compute garbage from the bias alone; re-zero them.
                if s == 0:
                    nc.vector.memset(Cn[:, Bp:NB], 0.0)
                    nc.vector.memset(Hn[0:65, Bp:NB], 0.0)
                if s == 1:
                    nc.vector.memset(Cn[:, 2 * Bp:NB], 0.0)
                    nc.vector.memset(Hn[0:65, 2 * Bp:NB], 0.0)

                # stage top-layer h (timestep t = s - 2): on the DVE right
                # after v4 (in-order, so it never adds a wait to v4 or the
                # next step's matmuls)
                if s >= 2:
                    t = s - 2
                    c = t // CHUNK_T
                    tp = t % CHUNK_T
                    st = stage[c % 2]
                    nc.vector.tensor_copy(
                        st[0:65, Bp * tp:Bp * (tp + 1)],
                        Hn[0:65, 2 * Bp:NB],
                    )
                    if tp == CHUNK_T - 1 or t == T - 1:
                        nt = tp + 1
                        rows = Bp * nt
                        yps = yp.tile([Bp * CHUNK_T, UNITS], f32, name="yps")
                        nc.tensor.matmul(
                            yps[0:rows, :], st[:, 0:rows], WD,
                            start=True, stop=True,
                        )
                        ysb = work.tile([Bp * CHUNK_T, UNITS], f32, name="ysb")
                        nc.scalar.copy(ysb[0:rows, :], yps[0:rows, :])
                        nc.sync.dma_start(
                            y_d[:, CHUNK_T * c:CHUNK_T * c + nt, :]
                            .rearrange("b t d -> t b d"),
                            ysb[0:rows, :],
                        )

                Cc = Cn
    nc.finalize()
    return nc


def _prep_weights(W, U, b, Wd, bd):
    """Permute gates (i,f,g,o) -> (f,g,i,o), scale g-columns by 2, fold
    biases into an extra contraction row; pack into one [66, 585] tensor."""
    perm = np.concatenate([np.arange(65, 130), np.arange(130, 195),
                           np.arange(0, 65), np.arange(195, 260)])
    gscale = np.concatenate([np.ones(65, np.float32),
                             np.full(65, 2.0, np.float32),
                             np.ones(130, np.float32)])
    import ml_dtypes
    Wp = (W[:, perm] * gscale).astype(np.float32)
    Up = (U[:, perm] * gscale).astype(np.float32)
    bp = (b[perm] * gscale).astype(np.float32)
    WALL = np.zeros((66, 585), np.float32)
    WALL[0:65, 0:260] = Wp
    WALL[65, 0:260] = bp
    WALL[0:65, 260:520] = Up
    WALL[0:65, 520:585] = Wd.astype(np.float32)
    WALL[65, 520:585] = bd.astype(np.float32)
    return np.ascontiguousarray(WALL.astype(ml_dtypes.bfloat16))


def _prep_xT(xs, T):
    """xs [Bp, T, 65] float32 -> bf16 feature-major [66, Bp*(T+2)]."""
    import ml_dtypes
    Bp = xs.shape[0]
    S = T + 2
    xTc = np.zeros((66, Bp * S), np.float32)
    xTc[65, :] = 1.0
    xTc[0:65, 0:T * Bp] = xs.transpose(1, 0, 2).reshape(T * Bp, 65).T
    return np.ascontiguousarray(xTc.astype(ml_dtypes.bfloat16))


_PROG = None

# test-harness knobs (harness calls kernel() with defaults)
TRACE = False
TRACE_KWARGS = {}
LAST_RESULT = None


def _get_program():
    global _PROG
    if _PROG is None:
        _PROG = _build_program(T_FULL, BP)
    return _PROG


def kernel(x, W, U, b, Wd, bd):
    from concourse.bass_utils import run_bass_kernel_spmd

    x = np.asarray(x, np.float32)
    B, T, D = x.shape
    assert (T, D) == (T_FULL, UNITS)

    WALL = _prep_weights(
        np.asarray(W, np.float32), np.asarray(U, np.float32),
        np.asarray(b, np.float32), np.asarray(Wd, np.float32),
        np.asarray(bd, np.float32),
    )

    S = T + 2
    xpad = np.zeros((NCORES * BP, T, D), np.float32)
    xpad[:B] = x

    in_maps = []
    for c in range(NCORES):
        xs = xpad[c * BP:(c + 1) * BP]
        in_maps.append({"xT": _prep_xT(xs, T), "WALL": WALL})

    nc = _get_program()
    res = run_bass_kernel_spmd(nc, in_maps, list(range(NCORES)),
                               trace=TRACE, **TRACE_KWARGS)
    global LAST_RESULT
    LAST_RESULT = res
    y = np.concatenate([np.asarray(res.results[c]["y"])
                        for c in range(NCORES)], axis=0)[:B]
    return np.ascontiguousarray(y.astype(np.float32))
